# revision 1
# baseline (speedup 1.0000x reference)
"""Paged-KV GQA decode attention on 8 TRN2 NeuronCores.

Strategy (data-parallel over flattened token tiles):
  * Host: resolve the paged cache (block_tables is a disjoint contiguous
    arange layout -> zero-copy reshape; general gather fallback otherwise),
    apply the store_kvcache update, slice each sequence's valid prefix
    [0, ctx_len), pad to 128-token tiles, and pack the global tile list.
  * The global tile stream is split contiguously across the 8 cores
    (perfect +-1 tile balance). Per tile the device computes, for each of
    the 8 KV heads, scoresT = K_tile^T @ qT (PE, stationary = K^T so scores
    land transposed [s, q]), w = exp(scoresT) (ACT, no max subtraction
    needed: |scores| <= ~6), o_tile = V_tile^T @ w (PE), and
    l_tile = ones^T @ w (PE). Per-tile unnormalized (o, l) go back to HBM.
  * Host: sum (o, l) over each sequence's tiles, subtract the exp(0)=1
    contribution of the zero-padded slots from l, divide, transpose.

Layouts are pre-transposed on the host so every device DMA is a fully
contiguous block and the PE never needs an on-chip transpose:
  kT:  [N_t, D=128, KVH*128]   (d, kh*128+s)   K^T stationary tiles
  v:   [N_t, 128,  KVH*128]    (s, kh*128+d)   V stationary tiles
  qT:  [N_t, D=128, H=32]      (d, kh*4+j)     pre-scaled by 1/sqrt(D)
"""

import math
import os

import numpy as np

B, H, KVH, D = 32, 32, 8, 128
G = H // KVH
BLOCK_SIZE = 16
MAX_BLOCKS = 256
NUM_BLOCKS = B * MAX_BLOCKS
MAX_KV = MAX_BLOCKS * BLOCK_SIZE
SCALE = 0.08838834764831845
NCORES = 8
TILE = 128

# compute/storage dtype for K/V/q/w on device ("float32" or "bfloat16")
KV_DTYPE = os.environ.get("BASS_KV_DTYPE", "float32")

LAST_RESULT = None  # BassKernelResults of the most recent run (for test.py)

_NC_CACHE = {}


def _install_trace_shim():
    """Register the axon NTFF profile hook (missing from the stub antenv) and
    stub the S3 artifact upload, so trace=True yields exec_time_ns."""
    import sys
    import types

    if "antenv.axon_hooks" not in sys.modules:
        mod = types.ModuleType("antenv.axon_hooks")
        _hook = [None]
        mod.set_axon_ntff_profile_hook = lambda h: _hook.__setitem__(0, h)
        mod.get_axon_ntff_profile_hook = lambda: _hook[0]
        sys.modules["antenv.axon_hooks"] = mod
        import antenv

        antenv.axon_hooks = mod
    from antenv.axon_hooks import (
        get_axon_ntff_profile_hook,
        set_axon_ntff_profile_hook,
    )

    if get_axon_ntff_profile_hook() is None:
        try:
            from trn_agent_boot.trn_boot import _ntff_profile_via_ctypes

            set_axon_ntff_profile_hook(
                _ntff_profile_via_ctypes("/opt/axon/libaxon_pjrt.so")
            )
        except Exception:
            pass
    import concourse.bass_utils as bu

    bu.upload_artifacts = lambda tmpdir: f"file://{tmpdir}"


def _build_nc(n_t: int, dt_name: str):
    import concourse.mybir as mybir
    import concourse.tile as tile
    from concourse import bacc

    key = (n_t, dt_name)
    if key in _NC_CACHE:
        return _NC_CACHE[key]

    DT = getattr(mybir.dt, dt_name)
    F32 = mybir.dt.float32

    nc = bacc.Bacc("TRN2", target_bir_lowering=False, num_devices=NCORES)
    kT = nc.dram_tensor("kT", [n_t, D, KVH * TILE], DT, kind="ExternalInput")
    vv = nc.dram_tensor("v", [n_t, TILE, KVH * D], DT, kind="ExternalInput")
    qT = nc.dram_tensor("qT", [n_t, D, H], DT, kind="ExternalInput")
    o = nc.dram_tensor("o", [n_t, D, H], F32, kind="ExternalOutput")
    ll = nc.dram_tensor("l", [n_t, 1, H], F32, kind="ExternalOutput")

    with tile.TileContext(nc) as tc:
        with (
            tc.tile_pool(name="consts", bufs=1) as consts,
            tc.tile_pool(name="kv", bufs=4) as kv_pool,
            tc.tile_pool(name="qp", bufs=4) as q_pool,
            tc.tile_pool(name="wt", bufs=4) as wt_pool,
            tc.tile_pool(name="outs", bufs=4) as out_pool,
            tc.tile_pool(name="ps_sc", bufs=2, space="PSUM") as ps_sc,
            tc.tile_pool(name="ps_o", bufs=2, space="PSUM") as ps_o,
            tc.tile_pool(name="ps_l", bufs=2, space="PSUM") as ps_l,
        ):
            ones = consts.tile([TILE, 1], DT)
            nc.vector.memset(ones, 1.0)
            for t in range(n_t):
                kT_t = kv_pool.tile([D, KVH * TILE], DT, tag="kT")
                nc.sync.dma_start(out=kT_t, in_=kT[t])
                v_t = kv_pool.tile([TILE, KVH * D], DT, tag="v")
                nc.sync.dma_start(out=v_t, in_=vv[t])
                qT_t = q_pool.tile([D, H], DT)
                nc.sync.dma_start(out=qT_t, in_=qT[t])

                sc = ps_sc.tile([TILE, H], F32)
                for kh in range(KVH):
                    nc.tensor.matmul(
                        sc[:, kh * G:(kh + 1) * G],
                        lhsT=kT_t[:, kh * TILE:(kh + 1) * TILE],
                        rhs=qT_t[:, kh * G:(kh + 1) * G],
                        start=True,
                        stop=True,
                    )
                w_t = wt_pool.tile([TILE, H], DT)
                nc.scalar.activation(w_t, sc, mybir.ActivationFunctionType.Exp)

                o_ps = ps_o.tile([D, H], F32)
                for kh in range(KVH):
                    nc.tensor.matmul(
                        o_ps[:, kh * G:(kh + 1) * G],
                        lhsT=v_t[:, kh * D:(kh + 1) * D],
                        rhs=w_t[:, kh * G:(kh + 1) * G],
                        start=True,
                        stop=True,
                    )
                l_ps = ps_l.tile([1, H], F32)
                nc.tensor.matmul(l_ps, lhsT=ones, rhs=w_t, start=True, stop=True)

                o_sb = out_pool.tile([D, H], F32, tag="o")
                nc.vector.tensor_copy(o_sb, o_ps)
                l_sb = out_pool.tile([1, H], F32, tag="l")
                nc.vector.tensor_copy(l_sb, l_ps)
                nc.sync.dma_start(out=o[t], in_=o_sb)
                nc.sync.dma_start(out=ll[t], in_=l_sb)
    nc.finalize()
    _NC_CACHE[key] = nc
    return nc


def kernel(q, k, v, k_cache, v_cache, block_tables, context_lens, slot_mapping):
    global LAST_RESULT
    from concourse.bass_utils import run_bass_kernel_spmd

    trace = bool(os.environ.get("BASS_TRACE"))
    if trace:
        _install_trace_shim()

    q = np.asarray(q, dtype=np.float32)
    k = np.asarray(k, dtype=np.float32)
    v = np.asarray(v, dtype=np.float32)
    k_cache = np.asarray(k_cache)
    v_cache = np.asarray(v_cache)
    block_tables = np.asarray(block_tables)
    context_lens = np.asarray(context_lens).astype(np.int64)
    slot_mapping = np.asarray(slot_mapping).astype(np.int64)

    # --- resolve paged layout -------------------------------------------------
    if np.array_equal(block_tables.ravel(), np.arange(NUM_BLOCKS, dtype=np.int64)):
        k_seq = k_cache.reshape(B, MAX_KV, KVH, D)  # zero-copy view
        v_seq = v_cache.reshape(B, MAX_KV, KVH, D)
        flat_pos = slot_mapping  # slot index == b*MAX_KV + pos under arange tables
    else:  # general fallback: true gather (slow, but correct for any table)
        k_seq = k_cache[block_tables].reshape(B, MAX_KV, KVH, D)
        v_seq = v_cache[block_tables].reshape(B, MAX_KV, KVH, D)
        # find logical position of each written slot within its sequence
        blk = slot_mapping // BLOCK_SIZE
        off = slot_mapping % BLOCK_SIZE
        flat_pos = np.empty(B, np.int64)
        for b in range(B):
            tb = np.where(block_tables[b] == blk[b])[0][0]
            flat_pos[b] = b * MAX_KV + tb * BLOCK_SIZE + off[b]

    # --- tile map -------------------------------------------------------------
    ctx = context_lens.astype(np.int64)
    n_t_seq = [int(math.ceil(int(c) / TILE)) for c in ctx]
    seq_tile_start = np.concatenate([[0], np.cumsum(n_t_seq)]).astype(np.int64)
    g_tiles = int(seq_tile_start[-1])
    n_t = (g_tiles + NCORES - 1) // NCORES
    g_pad = n_t * NCORES

    if KV_DTYPE == "bfloat16":
        import ml_dtypes

        dt_np = ml_dtypes.bfloat16
    else:
        dt_np = np.float32

    kT_g = np.zeros((g_pad, D, KVH * TILE), dt_np)
    v_g = np.zeros((g_pad, TILE, KVH * D), dt_np)
    qT_g = np.zeros((g_pad, D, H), dt_np)

    for b in range(B):
        c = int(ctx[b])
        t0 = int(seq_tile_start[b])
        nt = n_t_seq[b]
        kb = np.zeros((nt * TILE, KVH, D), np.float32)
        vb = np.zeros((nt * TILE, KVH, D), np.float32)
        kb[:c] = k_seq[b, :c]
        vb[:c] = v_seq[b, :c]
        # store_kvcache: new token for seq b lands at flat_pos[b] % MAX_KV
        p = int(flat_pos[b] - b * MAX_KV)
        if 0 <= p < c:
            kb[p] = k[b]
            vb[p] = v[b]
        # [s, kh, d] -> [t, d, kh, s] and [t, s, kh, d]
        kt = kb.reshape(nt, TILE, KVH, D).transpose(0, 3, 2, 1)
        kT_g[t0:t0 + nt] = kt.reshape(nt, D, KVH * TILE).astype(dt_np)
        v_g[t0:t0 + nt] = vb.reshape(nt, TILE, KVH * D).astype(dt_np)
        qT_g[t0:t0 + nt] = (q[b].T * SCALE).astype(dt_np)[None]

    in_maps = [
        {
            "kT": kT_g[c0 * n_t:(c0 + 1) * n_t],
            "v": v_g[c0 * n_t:(c0 + 1) * n_t],
            "qT": qT_g[c0 * n_t:(c0 + 1) * n_t],
        }
        for c0 in range(NCORES)
    ]

    nc = _build_nc(n_t, KV_DTYPE)
    res = run_bass_kernel_spmd(
        nc, in_maps, core_ids=list(range(NCORES)), trace=trace
    )
    LAST_RESULT = res

    o_all = np.concatenate([res.results[c]["o"] for c in range(NCORES)], axis=0)
    l_all = np.concatenate([res.results[c]["l"] for c in range(NCORES)], axis=0)

    out = np.empty((B, H, D), np.float32)
    for b in range(B):
        t0 = int(seq_tile_start[b])
        nt = n_t_seq[b]
        o_b = o_all[t0:t0 + nt].sum(axis=0)          # [D, H]
        l_b = l_all[t0:t0 + nt].sum(axis=0)[0]       # [H]
        l_b = l_b - (nt * TILE - int(ctx[b]))        # remove exp(0) pad terms
        out[b] = (o_b / l_b).T
    return out


# revision 2
# speedup vs baseline: 1.8133x; 1.8133x over previous
"""Paged-KV GQA decode attention on 8 TRN2 NeuronCores.

Strategy (data-parallel over flattened token tiles):
  * Host: resolve the paged cache (block_tables is a disjoint contiguous
    arange layout -> zero-copy reshape; general gather fallback otherwise),
    apply the store_kvcache update, slice each sequence's valid prefix
    [0, ctx_len), pad to 128-token tiles, and pack the global tile list.
  * The global tile stream is split contiguously across the 8 cores
    (perfect +-1 tile balance). Per tile the device computes, for each of
    the 8 KV heads, scoresT = K_tile^T @ qT (PE, stationary = K^T so scores
    land transposed [s, q]), w = exp(scoresT) (ACT, no max subtraction
    needed: |scores| <= ~6), o_tile = V_tile^T @ w (PE), and
    l_tile = ones^T @ w (PE). Per-tile unnormalized (o, l) go back to HBM.
  * Host: sum (o, l) over each sequence's tiles, subtract the exp(0)=1
    contribution of the zero-padded slots from l, divide, transpose.

Layouts are pre-transposed on the host so every device DMA is one fully
contiguous block per tile and the PE never needs an on-chip transpose.
Per-tile input row layout (128 partitions x 2080 bf16):
  cols [0,1024):     K^T   (partition=d, col=kh*128+s)
  cols [1024,2048):  V     (partition=s, col=kh*128+d)
  cols [2048,2080):  q^T   (partition=d, col=kh*4+j), pre-scaled by 1/sqrt(D)
Per-tile output row layout (128 partitions x 64 f32):
  cols [0,32):  o_tile (partition=d, col=kh*4+j), unnormalized
  row 0, cols [32,64):  l_tile (sum of exp weights per (kh,j))
"""

import math
import os

import numpy as np

B, H, KVH, D = 32, 32, 8, 128
G = H // KVH
BLOCK_SIZE = 16
MAX_BLOCKS = 256
NUM_BLOCKS = B * MAX_BLOCKS
MAX_KV = MAX_BLOCKS * BLOCK_SIZE
SCALE = 0.08838834764831845
NCORES = 8
TILE = 128

KV_DTYPE = os.environ.get("BASS_KV_DTYPE", "bfloat16")

X_COLS = KVH * TILE + KVH * D + H  # 2080
O_OFF = 0
L_OFF = H  # in the [128, 64] output tile, l lives at row 0, cols [32,64)

LAST_RESULT = None  # BassKernelResults of the most recent run (for test.py)

_NC_CACHE = {}


def _install_trace_shim():
    """Register the axon NTFF profile hook (missing from the stub antenv) and
    stub the S3 artifact upload, so trace=True yields exec_time_ns."""
    import sys
    import types

    if "antenv.axon_hooks" not in sys.modules:
        mod = types.ModuleType("antenv.axon_hooks")
        _hook = [None]
        mod.set_axon_ntff_profile_hook = lambda h: _hook.__setitem__(0, h)
        mod.get_axon_ntff_profile_hook = lambda: _hook[0]
        sys.modules["antenv.axon_hooks"] = mod
        import antenv

        antenv.axon_hooks = mod
    from antenv.axon_hooks import (
        get_axon_ntff_profile_hook,
        set_axon_ntff_profile_hook,
    )

    if get_axon_ntff_profile_hook() is None:
        try:
            from trn_agent_boot.trn_boot import _ntff_profile_via_ctypes

            set_axon_ntff_profile_hook(
                _ntff_profile_via_ctypes("/opt/axon/libaxon_pjrt.so")
            )
        except Exception:
            pass
    import concourse.bass_utils as bu

    bu.upload_artifacts = lambda tmpdir: f"file://{tmpdir}"


def _build_nc(n_t: int, dt_name: str):
    import concourse.mybir as mybir
    import concourse.tile as tile
    from concourse import bacc

    key = (n_t, dt_name)
    if key in _NC_CACHE:
        return _NC_CACHE[key]

    DT = getattr(mybir.dt, dt_name)
    F32 = mybir.dt.float32
    KOFF, VOFF, QOFF = 0, KVH * TILE, 2 * KVH * TILE

    nc = bacc.Bacc("TRN2", target_bir_lowering=False, num_devices=NCORES)
    x = nc.dram_tensor("x", [n_t, TILE, X_COLS], DT, kind="ExternalInput")
    y = nc.dram_tensor("y", [n_t, TILE, 2 * H], F32, kind="ExternalOutput")

    with tile.TileContext(nc) as tc:
        with (
            tc.tile_pool(name="consts", bufs=1) as consts,
            tc.tile_pool(name="xp", bufs=4) as x_pool,
            tc.tile_pool(name="wt", bufs=4) as wt_pool,
            tc.tile_pool(name="outs", bufs=4) as out_pool,
            tc.tile_pool(name="ps_sc", bufs=3, space="PSUM") as ps_sc,
            tc.tile_pool(name="ps_o", bufs=3, space="PSUM") as ps_o,
            tc.tile_pool(name="ps_l", bufs=2, space="PSUM") as ps_l,
        ):
            ones = consts.tile([TILE, 1], DT)
            nc.vector.memset(ones, 1.0)
            for t in range(n_t):
                x_t = x_pool.tile([TILE, X_COLS], DT)
                nc.sync.dma_start(out=x_t, in_=x[t])

                sc = ps_sc.tile([TILE, H], F32)
                for kh in range(KVH):
                    nc.tensor.matmul(
                        sc[:, kh * G:(kh + 1) * G],
                        lhsT=x_t[:, KOFF + kh * TILE:KOFF + (kh + 1) * TILE],
                        rhs=x_t[:, QOFF + kh * G:QOFF + (kh + 1) * G],
                        start=(kh == 0),
                        stop=(kh == KVH - 1),
                    )
                w_t = wt_pool.tile([TILE, H], DT)
                nc.scalar.activation(w_t, sc, mybir.ActivationFunctionType.Exp)

                o_ps = ps_o.tile([D, H], F32)
                for kh in range(KVH):
                    nc.tensor.matmul(
                        o_ps[:, kh * G:(kh + 1) * G],
                        lhsT=x_t[:, VOFF + kh * D:VOFF + (kh + 1) * D],
                        rhs=w_t[:, kh * G:(kh + 1) * G],
                        start=(kh == 0),
                        stop=(kh == KVH - 1),
                    )
                l_ps = ps_l.tile([1, H], F32)
                nc.tensor.matmul(l_ps, lhsT=ones, rhs=w_t, start=True, stop=True)

                y_sb = out_pool.tile([TILE, 2 * H], F32)
                nc.vector.tensor_copy(y_sb[:, :H], o_ps)
                nc.vector.tensor_copy(y_sb[0:1, H:], l_ps)
                nc.sync.dma_start(out=y[t], in_=y_sb)
    nc.finalize()
    _NC_CACHE[key] = nc
    return nc


def kernel(q, k, v, k_cache, v_cache, block_tables, context_lens, slot_mapping):
    global LAST_RESULT
    from concourse.bass_utils import run_bass_kernel_spmd

    trace = bool(os.environ.get("BASS_TRACE"))
    if trace:
        _install_trace_shim()

    q = np.asarray(q, dtype=np.float32)
    k = np.asarray(k, dtype=np.float32)
    v = np.asarray(v, dtype=np.float32)
    k_cache = np.asarray(k_cache)
    v_cache = np.asarray(v_cache)
    block_tables = np.asarray(block_tables)
    context_lens = np.asarray(context_lens).astype(np.int64)
    slot_mapping = np.asarray(slot_mapping).astype(np.int64)

    # --- resolve paged layout -------------------------------------------------
    if np.array_equal(block_tables.ravel(), np.arange(NUM_BLOCKS, dtype=np.int64)):
        k_seq = k_cache.reshape(B, MAX_KV, KVH, D)  # zero-copy view
        v_seq = v_cache.reshape(B, MAX_KV, KVH, D)
        flat_pos = slot_mapping  # slot index == b*MAX_KV + pos under arange tables
    else:  # general fallback: true gather (slow, but correct for any table)
        k_seq = k_cache[block_tables].reshape(B, MAX_KV, KVH, D)
        v_seq = v_cache[block_tables].reshape(B, MAX_KV, KVH, D)
        blk = slot_mapping // BLOCK_SIZE
        off = slot_mapping % BLOCK_SIZE
        flat_pos = np.empty(B, np.int64)
        for b in range(B):
            tb = np.where(block_tables[b] == blk[b])[0][0]
            flat_pos[b] = b * MAX_KV + tb * BLOCK_SIZE + off[b]

    # --- tile map -------------------------------------------------------------
    ctx = context_lens.astype(np.int64)
    n_t_seq = [int(math.ceil(int(c) / TILE)) for c in ctx]
    seq_tile_start = np.concatenate([[0], np.cumsum(n_t_seq)]).astype(np.int64)
    g_tiles = int(seq_tile_start[-1])
    n_t = (g_tiles + NCORES - 1) // NCORES
    g_pad = n_t * NCORES

    if KV_DTYPE == "bfloat16":
        import ml_dtypes

        dt_np = ml_dtypes.bfloat16
    else:
        dt_np = np.float32

    x_g = np.zeros((g_pad, TILE, X_COLS), dt_np)
    KOFF, VOFF, QOFF = 0, KVH * TILE, 2 * KVH * TILE

    for b in range(B):
        c = int(ctx[b])
        t0 = int(seq_tile_start[b])
        nt = n_t_seq[b]
        kb = np.zeros((nt * TILE, KVH, D), np.float32)
        vb = np.zeros((nt * TILE, KVH, D), np.float32)
        kb[:c] = k_seq[b, :c]
        vb[:c] = v_seq[b, :c]
        # store_kvcache: new token for seq b lands at flat_pos[b] % MAX_KV
        p = int(flat_pos[b] - b * MAX_KV)
        if 0 <= p < c:
            kb[p] = k[b]
            vb[p] = v[b]
        # K^T tiles: [s, kh, d] -> [t, d, kh, s]
        kt = kb.reshape(nt, TILE, KVH, D).transpose(0, 3, 2, 1)
        x_g[t0:t0 + nt, :, KOFF:VOFF] = kt.reshape(nt, D, KVH * TILE).astype(dt_np)
        # V tiles: [t, s, kh*d]
        x_g[t0:t0 + nt, :, VOFF:QOFF] = vb.reshape(nt, TILE, KVH * D).astype(dt_np)
        x_g[t0:t0 + nt, :, QOFF:] = (q[b].T * SCALE).astype(dt_np)[None]

    in_maps = [{"x": x_g[c0 * n_t:(c0 + 1) * n_t]} for c0 in range(NCORES)]

    nc = _build_nc(n_t, KV_DTYPE)
    res = run_bass_kernel_spmd(
        nc, in_maps, core_ids=list(range(NCORES)), trace=trace
    )
    LAST_RESULT = res

    y_all = np.concatenate([res.results[c]["y"] for c in range(NCORES)], axis=0)

    out = np.empty((B, H, D), np.float32)
    for b in range(B):
        t0 = int(seq_tile_start[b])
        nt = n_t_seq[b]
        o_b = y_all[t0:t0 + nt, :, :H].sum(axis=0)       # [D, H]
        l_b = y_all[t0:t0 + nt, 0, H:].sum(axis=0)       # [H]
        l_b = l_b - (nt * TILE - int(ctx[b]))            # remove exp(0) pad terms
        out[b] = (o_b / l_b).T
    return out
